# revision 28
# baseline (speedup 1.0000x reference)
"""Trainium2 Bass kernel for nn_ATT_14972255993877 (dense_transformer).

Reference computation (B=4096, NK=128, U=256):
    q = query @ Wq.T + bq                      # (B,U)
    k = keys @ Wk.T + bk                       # (B,NK,U)
    scores = einsum('bu,bnu->bn', q, k)/16     # (B,NK)
    p = softmax(scores, -1)
    v = keys @ Wv.T + bv
    ctx = einsum('bn,bnu->bu', p, v)
    out = relu(concat([ctx, query], -1) @ Wf.T + bf)

Algebraic restructuring (exact):
  * The per-key projections fold out:
      scores[b,n] = keys[b,n,:] . qt[b,:] + const(b)   with
      qt = query @ (Wq.T @ Wk)/16 + (bq @ Wk)/16;  const(b) = (q.bk)/16
    const(b) is constant over n -> cancels in softmax -> dropped.
  * softmax weights sum to 1, so with pk[b,:] = sum_n p[b,n]*keys[b,n,:]:
      ctx = pk @ Wv.T + bv
      out = relu(pk @ (Wv.T Wf1.T) + query @ Wf2.T + (Wf1 bv + bf))
    where Wf = [Wf1 | Wf2] column split.
  Device work: 2 streaming passes over keys + small matmuls (~3 GFLOP).

Sharding: data-parallel over B across 8 cores (512 rows/core); keys are
host-permuted to [n, b_local, v] so every keys DMA has 8KB-contiguous HBM
runs (128 descriptors/MB instead of 1024).

Per-core device plan (keys tile layout [n=128 partitions, v free]):
  stage A (PE):  qt[128b, 256v] = queryT-chunks.T @ Wqk + ones x bqk (fp32);
                 qt split into hi/lo fp32r halves (hi = fp32r(qt), lo =
                 qt - hi) and one strided DMA spreads each onto partitions
                 {0,32,64,96} as 32 row-slots for legal matmul base partitions.
  pass 1:        per pair of b: one K=1 fp32r outer-product per half
                 accumulates ones x qt_row into a PSUM broadcast (exactly
                 reconstructing fp32 qt); per b, one DVE
                 scalar_tensor_tensor(keys[b] * bcast, accum_out) emits the
                 [128n, 1] scores column in a single instruction.
  softmax:       ACT Exp into fp16 per 8-b batch (scores are O(1) so the
                 max-shift cancels exactly and is skipped); Z row via fp16
                 ones-matvec per batch; Z row -> column via K=1 fp32 matmul;
                 DVE reciprocal.
  pass 2 (PE):   per b: two fp16 matmuls with keys v-chunk [128n, 128v] as
                 the stationary and the fp16 exp column as the mover write
                 the (unnormalized) pkT[:, b] column straight into PSUM.
                 (keys -> fp16 copies are produced on the otherwise-idle ACT.)
  stage J (PE):  J1 = pkT-chunks.T @ Wpf (fp32); ACT copies J1 out of PSUM
                 scaled by 1/Z (per-partition, b on partitions);
                 J2 = queryT.T @ Wf2T + ones x bfull (fp32);
                 out = relu(J1 + J2) on DVE; DMA out.

Measured on hw: relative L2 error ~1.6e-5 (absmax ~4e-5); per-body exec
~130-180 us/core (paired body-repeat differential over PJRT).
"""

import sys

sys.path.insert(0, "/opt/trn_rl_repo")

import numpy as np

import concourse.bass as bass  # noqa: F401  (registers types)
import concourse.bacc as bacc
import concourse.tile as tile
import concourse.mybir as mybir
from concourse.bass_utils import run_bass_kernel_spmd

B, NK, U = 4096, 128, 256
N_CORES = 8
BL = B // N_CORES          # 512 batch rows per core
NT = BL // 128             # 4 b-tiles per core
NBATCH = 16                # batches of 8 b's per b-tile
dt = mybir.dt.float32
# float32r: same 32-bit container, PE matmul runs 4x faster (1 cycle/row vs 4
# for strict fp32) at TF32-like precision (~1e-4 rel err measured on hw).
dtr = mybir.dt.float32r
# fp16 for the per-b pk matvecs: fp32r requires even N + dst partition 0,
# which an N=1 matvec can't satisfy; fp16 runs 1 cycle/row at ~2.4e-4 rounding.
f16 = mybir.dt.float16
F32 = np.float32

_NC_CACHE = None


def build_nc(repeat=1):
    AT = mybir.AluOpType
    AF = mybir.ActivationFunctionType

    nc = bacc.Bacc("TRN2", target_bir_lowering=False, debug=False,
                   enable_asserts=False, num_devices=N_CORES)
    # keys pre-permuted on host to [n, b_local, v] so each SBUF tile DMA has
    # 8KB-contiguous HBM runs (128 descriptors instead of 1024).
    keys_d = nc.dram_tensor("keys", [NK, BL, U], dt, kind="ExternalInput").ap()
    qT_d = nc.dram_tensor("qT", [U, BL], dt, kind="ExternalInput").ap()
    Wqk_d = nc.dram_tensor("Wqk", [U, U], dt, kind="ExternalInput").ap()
    bqk_d = nc.dram_tensor("bqk", [1, U], dt, kind="ExternalInput").ap()
    Wpf_d = nc.dram_tensor("Wpf", [U, U], dt, kind="ExternalInput").ap()
    Wf2T_d = nc.dram_tensor("Wf2T", [U, U], dt, kind="ExternalInput").ap()
    bfull_d = nc.dram_tensor("bfull", [1, U], dt, kind="ExternalInput").ap()
    out_d = nc.dram_tensor("out", [BL, U], dt, kind="ExternalOutput").ap()


    with tile.TileContext(nc) as tc:
        with (
            tc.tile_pool(name="const", bufs=1) as consts,
            tc.tile_pool(name="keys", bufs=5) as p_keys,
            tc.tile_pool(name="k16", bufs=4) as p_k16,
            tc.tile_pool(name="qr", bufs=1) as p_qr,
            tc.tile_pool(name="qt", bufs=2) as p_qt,
            tc.tile_pool(name="sc", bufs=3) as p_sc,
            tc.tile_pool(name="ex", bufs=18) as p_ex,
            tc.tile_pool(name="pkT", bufs=4) as p_pkT,
            tc.tile_pool(name="js", bufs=2) as p_js,
            tc.tile_pool(name="scr", bufs=2) as p_scr,
            tc.tile_pool(name="z", bufs=4) as p_z,
            tc.tile_pool(name="outp", bufs=2) as p_out,
            tc.tile_pool(name="ps_bc", bufs=4, space="PSUM") as ps_bc,
            tc.tile_pool(name="ps_pkT", bufs=2, space="PSUM") as ps_pkT,
            tc.tile_pool(name="ps_stage", bufs=2, space="PSUM") as ps_stage,
        ):
            ones_blk = consts.tile([128, 128], dt, tag="ones_blk")
            nc.gpsimd.memset(ones_blk[:], 1.0)
            ones_r = consts.tile([128, 128], dtr, tag="ones_r")
            nc.scalar.copy(ones_r[:], ones_blk[:])
            ones16 = consts.tile([128, 1], f16, tag="ones16")
            nc.gpsimd.memset(ones16[:], 1.0)

            def load_chunks(src_ap, tagbase):
                ts = []
                for c in range(2):
                    t = consts.tile([128, U], dt, tag=f"{tagbase}{c}")
                    nc.sync.dma_start(t[:], src_ap[c * 128:(c + 1) * 128, :])
                    ts.append(t)
                return ts

            wqk = load_chunks(Wqk_d, "wqk")
            wpf = load_chunks(Wpf_d, "wpf")
            wf2t = load_chunks(Wf2T_d, "wf2t")
            bqk_s = consts.tile([1, U], dt, tag="bqk")
            nc.sync.dma_start(bqk_s[:], bqk_d[:, :])
            bfull_s = consts.tile([1, U], dt, tag="bfull")
            nc.sync.dma_start(bfull_s[:], bfull_d[:, :])
            qT = []
            for c in range(2):
                t = consts.tile([128, BL], dt, tag=f"qTs{c}")
                nc.sync.dma_start(t[:], qT_d[c * 128:(c + 1) * 128, :])
                qT.append(t)

            def emit_stage_a(bt):
                # qt[b,v] = query @ Wqk + bqk, then one DMA spreads the 128 qt
                # rows onto partitions {0,32,64,96} (32 b-slots each) so the
                # broadcast matmuls can read them at legal base partitions.
                b0 = bt * 128
                psA = ps_stage.tile([128, U], dt, tag="stage")
                nc.tensor.matmul(psA[:], qT[0][:, b0:b0 + 128], wqk[0][:],
                                 start=True, stop=False)
                nc.tensor.matmul(psA[:], qT[1][:, b0:b0 + 128], wqk[1][:],
                                 start=False, stop=False)
                nc.tensor.matmul(psA[:], ones_blk[0:1, :], bqk_s[:],
                                 start=False, stop=True)
                # exact broadcast trick: qt = hi + lo with hi = fp32r(qt);
                # two accumulating fp32r outer products reconstruct qt
                # exactly in the fp32 PSUM broadcast.
                qt_hi = p_qt.tile([128, U], dtr, tag="qt_hi")
                nc.scalar.copy(qt_hi[:], psA[:])
                qt_lo = p_qt.tile([128, U], dtr, tag="qt_lo")
                nc.vector.scalar_tensor_tensor(
                    out=qt_lo[:], in0=psA[:], scalar=1.0,
                    in1=qt_hi[:].bitcast(dt),
                    op0=AT.mult, op1=AT.subtract)
                qr4h = p_qr.tile([128, 32, U], dtr, tag="qr4h")
                nc.gpsimd.dma_start(qr4h[0:128:32, :, :], qt_hi[:, :])
                qr4l = p_qr.tile([128, 32, U], dtr, tag="qr4l")
                nc.gpsimd.dma_start(qr4l[0:128:32, :, :], qt_lo[:, :])
                return qr4h, qr4l

            qr4_next = emit_stage_a(0)

            for t in range(NT * repeat):
                bt = t % NT
                b0 = bt * 128
                qr4h, qr4l = qr4_next

                pkT_ps = []
                for _c in range(2):
                    pkT_ps_c = ps_pkT.tile([128, 128], dt, tag="pkT_ps")
                    pkT_ps.append(pkT_ps_c)
                kts = [None] * NBATCH
                exps = [None] * NBATCH

                def emit_front(i, b0=b0, qr4h=qr4h, qr4l=qr4l,
                               kts=kts, exps=exps):
                    kt = p_keys.tile([128, 8, U], dt, tag="kt")
                    nc.sync.dma_start(
                        kt[:, :, :],
                        keys_d[:, b0 + i * 8: b0 + i * 8 + 8, :])
                    kt16 = p_k16.tile([128, 8, U], f16, tag="kt16")
                    nc.scalar.copy(kt16[:], kt[:])
                    kts[i] = (kt, kt16)
                    sc = p_sc.tile([128, 8], dt, tag="sc")
                    for pr in range(4):
                        bc = ps_bc.tile([128, 2, U], dt, tag="bc")
                        bl = i * 8 + pr * 2
                        jq = 32 * (bl // 32)
                        # one K=1 outer product broadcasts qt for 2 b's (N=512)
                        nc.tensor.matmul(bc[:, :, :],
                                         ones_r[jq:jq + 1, :],
                                         qr4h[jq:jq + 1,
                                              bl % 32: bl % 32 + 2, :],
                                         start=True, stop=False,
                                         tile_position=(jq, 0))
                        nc.tensor.matmul(bc[:, :, :],
                                         ones_r[jq:jq + 1, :],
                                         qr4l[jq:jq + 1,
                                              bl % 32: bl % 32 + 2, :],
                                         start=False, stop=True,
                                         tile_position=(jq, 0))
                        for h in range(2):
                            m = pr * 2 + h
                            scr = p_scr.tile([128, U], dt, tag="scr")
                            nc.vector.scalar_tensor_tensor(
                                out=scr[:], in0=kt[:, m, :],
                                scalar=1.0, in1=bc[:, h, :],
                                op0=AT.mult, op1=AT.mult,
                                accum_out=sc[:, m:m + 1])
                    ex = p_ex.tile([128, 8], f16, tag="ex")
                    nc.scalar.activation(ex[:], sc[:], AF.Exp)
                    exps[i] = ex

                def emit_pk(i, pkT_ps=pkT_ps, kts=kts, exps=exps):
                    # pkT[:, b] column = keys[b].T-contraction over n on the
                    # PE: lhsT = keys v-chunk [128n, 128v], rhs = exp column.
                    kt16 = kts[i][1]
                    ex = exps[i]
                    for m in range(8):
                        bl = i * 8 + m
                        for c in range(2):
                            nc.tensor.matmul(
                                pkT_ps[c][:, bl:bl + 1],
                                kt16[:, m, c * 128:(c + 1) * 128],
                                ex[:, m:m + 1],
                                start=True, stop=True)

                for i in range(NBATCH):
                    emit_front(i)
                    if i >= 2:
                        emit_pk(i - 2)
                emit_pk(NBATCH - 2)
                emit_pk(NBATCH - 1)

                # prologue of the next tile overlaps this tile's epilogue
                if t + 1 < NT * repeat:
                    qr4_next = emit_stage_a((bt + 1) % NT)

                # ---- softmax denominator: Z row -> transpose -> 1/Z column
                psZ = ps_stage.tile([1, 128], dt, tag="stage")
                for i in range(NBATCH):
                    nc.tensor.matmul(psZ[0:1, i * 8:(i + 1) * 8],
                                     ones16[:, 0:1], exps[i][:],
                                     start=True, stop=True)
                zr = p_z.tile([1, 128], dt, tag="zr")
                nc.scalar.copy(zr[:], psZ[0:1, :])
                # row -> column via K=1 matmul against [[1.0]]
                psZc = ps_stage.tile([128, 1], dt, tag="stage")
                nc.tensor.matmul(psZc[:, 0:1], zr[:, :], ones_blk[0:1, 0:1],
                                 start=True, stop=True)
                zc = p_z.tile([128, 1], dt, tag="zc")
                nc.vector.reciprocal(zc[:], psZc[:, 0:1])

                # ---- output matmuls, split so 1/Z applies per-partition on
                # the unnormalized pk partial (J1), then J2 adds query+bias
                pkT = []
                for c in range(2):
                    pt = p_pkT.tile([128, 128], dt, tag="pkT")
                    nc.scalar.copy(pt[:], pkT_ps[c][:])
                    pkT.append(pt)
                psJ1 = ps_stage.tile([128, U], dt, tag="stage")
                nc.tensor.matmul(psJ1[:], pkT[0][:], wpf[0][:],
                                 start=True, stop=False)
                nc.tensor.matmul(psJ1[:], pkT[1][:], wpf[1][:],
                                 start=False, stop=True)
                js = p_js.tile([128, U], dt, tag="js")
                nc.scalar.activation(js[:], psJ1[:], AF.Copy, scale=zc[:, 0:1])
                psJ2 = ps_stage.tile([128, U], dt, tag="stage")
                nc.tensor.matmul(psJ2[:], qT[0][:, b0:b0 + 128], wf2t[0][:],
                                 start=True, stop=False)
                nc.tensor.matmul(psJ2[:], qT[1][:, b0:b0 + 128], wf2t[1][:],
                                 start=False, stop=False)
                nc.tensor.matmul(psJ2[:], ones_blk[0:1, :], bfull_s[:],
                                 start=False, stop=True)
                out_s = p_out.tile([128, U], dt, tag="outp")
                nc.vector.tensor_tensor(out_s[:], js[:], psJ2[:], AT.add)
                nc.vector.tensor_scalar_max(out_s[:], out_s[:], 0.0)
                nc.scalar.dma_start(out_d[b0:b0 + 128, :], out_s[:])

    nc.compile()
    return nc


def _get_nc():
    global _NC_CACHE
    if _NC_CACHE is None:
        _NC_CACHE = build_nc()
    return _NC_CACHE


def prepare_in_maps(query, keys, Wq, bq, Wk, bk, Wv, bv, Wf, bf):
    query = np.asarray(query, F32)
    keys = np.asarray(keys, F32)
    Wq = np.asarray(Wq, F32)
    bq = np.asarray(bq, F32)
    Wk = np.asarray(Wk, F32)
    Wv = np.asarray(Wv, F32)
    bv = np.asarray(bv, F32)
    Wf = np.asarray(Wf, F32)
    bf = np.asarray(bf, F32)
    # bk shifts all scores of a row equally -> cancels in softmax; unused.

    scale = F32(1.0) / np.sqrt(F32(U))
    Wqk = (Wq.T @ Wk) * scale                    # [i, v]
    bqk = (bq @ Wk) * scale                      # [v]
    Wf1, Wf2 = Wf[:, :U], Wf[:, U:]
    Wpf = Wv.T @ Wf1.T                           # [v, o]
    Wf2T = np.ascontiguousarray(Wf2.T)           # [j, o]
    bfull = Wf1 @ bv + bf                        # [o]
    qT = np.ascontiguousarray(query.T)           # [i, B]

    # permute keys to [core, n, b_local, v] so device DMAs get 8KB-contiguous
    # HBM runs (the natural [b,n,v] layout would force 1KB descriptor runs)
    keys_nmaj = np.ascontiguousarray(
        keys.reshape(N_CORES, BL, NK, U).transpose(0, 2, 1, 3))

    in_maps = []
    for c in range(N_CORES):
        sl = slice(c * BL, (c + 1) * BL)
        in_maps.append({
            "keys": keys_nmaj[c],
            "qT": np.ascontiguousarray(qT[:, sl]),
            "Wqk": np.ascontiguousarray(Wqk.astype(F32)),
            "bqk": np.ascontiguousarray(bqk.astype(F32)).reshape(1, U),
            "Wpf": np.ascontiguousarray(Wpf.astype(F32)),
            "Wf2T": Wf2T.astype(F32),
            "bfull": np.ascontiguousarray(bfull.astype(F32)).reshape(1, U),
        })
    return in_maps


def run(in_maps, **kwargs):
    nc = _get_nc()
    return run_bass_kernel_spmd(nc, in_maps, list(range(N_CORES)), **kwargs)


def kernel(**inputs):
    in_maps = prepare_in_maps(**inputs)
    res = run(in_maps)
    out = np.concatenate([res.results[c]["out"] for c in range(N_CORES)], 0)
    return np.ascontiguousarray(out, dtype=np.float32)


if __name__ == "__main__":
    rng = np.random.default_rng(0)
    demo = {
        "query": rng.standard_normal((B, U), dtype=F32),
        "keys": rng.standard_normal((B, NK, U), dtype=F32),
        "Wq": rng.uniform(-1 / 16, 1 / 16, (U, U)).astype(F32),
        "bq": np.zeros(U, F32),
        "Wk": rng.uniform(-1 / 16, 1 / 16, (U, U)).astype(F32),
        "bk": np.zeros(U, F32),
        "Wv": rng.uniform(-1 / 16, 1 / 16, (U, U)).astype(F32),
        "bv": np.zeros(U, F32),
        "Wf": rng.uniform(-1 / 23, 1 / 23, (U, 2 * U)).astype(F32),
        "bf": np.zeros(U, F32),
    }
    out = kernel(**demo)
    print("kernel ran; out", out.shape, out.dtype, float(np.abs(out).max()))
